# revision 12
# baseline (speedup 1.0000x reference)
"""Distributed dot-product attention for 8 Trainium2 NeuronCores.

Problem: query/key/value [2048, 2, 32, 128] fp32, bool mask [2, 1, 2048, 2048]
(True -> masked). Output [2048, 2, 4096] fp32.

Strategy: shard batch*heads (64 of them) across 8 cores -> 8 heads/core.
Core c handles batch b = c//4, heads (c%4)*8 .. +8, so each core needs only
one batch's mask.

Per-head on-chip algorithm (all layouts prepared host-side):
  S^T[sk, sq]  = K^T.T @ Q^T            (bf16 matmuls, N=512, scores
                                         pre-scaled by 1/sqrt(128) via Q)
  P^T          = exp(S^T)               (ACT, bf16 out; no max-subtraction:
                                         scores ~ N(0,1), exp can't overflow)
  Pm^T         = P^T * keep^T           (DVE bf16 2x mode; keep = 1-mask,
                                         exp*0 = 0 reproduces the -10000 mask)
  U[sq, 0:129] = sum_sk Pm^T.T @ [V|1]  (bf16 matmuls accumulated in PSUM;
                                         col 128 = softmax denominator)
  O[sq, :]     = U[:, :128] / U[:, 128] (DVE reciprocal + per-partition mul)
"""

import math
import os

import numpy as np
import ml_dtypes

SQ, SK, B, NH, HN = 2048, 2048, 2, 32, 128
NCORES = 8
HPC = (B * NH) // NCORES  # heads per core = 8
NKT = SK // 128           # 16 sk tiles
NPAIR = NKT // 2          # 8 sk-tile pairs
NCH = SQ // 512           # 4 sq chunks of 512
VW = 130                  # V row padded: 128 V + 1 ones + 1 pad

LAST_RESULTS = None
_CACHE = {}
# Q/K matmul operand dtype: bf16 (split LDWEIGHTS+MM, FWL) measures ~70us
# faster than float32r (self-loading matmul serializes the weight load).
QK_BF16 = True

# Tuning knobs (shared by kernel() and test.py's loop-delta builds).
# out_div: "dve" = reciprocal+mul on DVE; "pool" = tensor_scalar divide on
# GPSIMD (frees DVE for the mask multiplies).
CFG = dict(s_bufs=2, pack_u=False, p_bufs=3, k_group=2, out_div="dve",
           mask_pool_frac=0.0,
           # timing-only diagnostic switches (break numerics, isolate engines)
           diag_no_u=False, diag_no_mask=False, diag_act_frac=1.0,
           diag_exp_const=False)


def _build_bass(
    repeat=1,
    loop_trips=None,
    s_bufs=None,
    pack_u=None,
    p_bufs=None,
    qk_bf16=False,
    k_group=None,
    out_div=None,
    mask_pool_frac=None,
):
    s_bufs = CFG["s_bufs"] if s_bufs is None else s_bufs
    pack_u = CFG["pack_u"] if pack_u is None else pack_u
    p_bufs = CFG["p_bufs"] if p_bufs is None else p_bufs
    k_group = CFG["k_group"] if k_group is None else k_group
    out_div = CFG["out_div"] if out_div is None else out_div
    mask_pool_frac = (
        CFG["mask_pool_frac"] if mask_pool_frac is None else mask_pool_frac
    )
    diag_no_u = CFG["diag_no_u"]
    diag_no_mask = CFG["diag_no_mask"]
    diag_act_frac = CFG["diag_act_frac"]
    diag_exp_const = CFG["diag_exp_const"]
    from contextlib import ExitStack

    import concourse.mybir as mybir
    import concourse.tile as tile
    from concourse import bacc

    f32 = mybir.dt.float32
    f32r = mybir.dt.float32r
    bf16 = mybir.dt.bfloat16
    Exp = mybir.ActivationFunctionType.Exp

    # Bacc (not plain Bass): its compile() pipeline splits multi-sem waits
    # into EventSemaphore instructions — TRN2 allows only 1 wait per
    # instruction and walrus rejects the raw Tile output otherwise.
    nc = bacc.Bacc("TRN2", target_bir_lowering=False)
    qk_dt = bf16 if qk_bf16 else f32r
    qTd = nc.dram_tensor("qT", [HPC, HN, SQ], qk_dt, kind="ExternalInput")
    kTd = nc.dram_tensor("kT", [HPC, HN, SK], qk_dt, kind="ExternalInput")
    vpd = nc.dram_tensor("vp", [HPC, 128, NKT * VW], bf16, kind="ExternalInput")
    mkd = nc.dram_tensor("keep", [NCH * NPAIR, 128, 1024], bf16, kind="ExternalInput")
    outd = nc.dram_tensor("out", [SQ, HPC, HN], f32, kind="ExternalOutput")

    with tile.TileContext(nc) as tc, ExitStack() as ctx:
        singles = ctx.enter_context(tc.tile_pool(name="singles", bufs=1))
        qkv = ctx.enter_context(tc.tile_pool(name="qkv", bufs=3))
        ppool = ctx.enter_context(tc.tile_pool(name="ppool", bufs=p_bufs))
        mpool = ctx.enter_context(tc.tile_pool(name="mpool", bufs=p_bufs))
        opool = ctx.enter_context(tc.tile_pool(name="opool", bufs=4))
        rpool = ctx.enter_context(tc.tile_pool(name="rpool", bufs=4))
        spsum = ctx.enter_context(tc.tile_pool(name="spsum", bufs=s_bufs, space="PSUM"))
        upsum = ctx.enter_context(
            tc.tile_pool(name="upsum", bufs=2 if pack_u else 4, space="PSUM")
        )

        keep_sb = singles.tile([128, NCH * NPAIR * 1024], bf16)

        # dependency-free dummy exp at t=0: walrus inserts the ~2.7us ACT
        # table load before the first ACTIVATE, so trigger it here where it
        # overlaps the startup DMAs instead of delaying the first real exp
        warm = singles.tile([128, 1], f32)
        nc.vector.memset(warm, 0.0)
        nc.scalar.activation(warm, warm, Exp)

        def load_keep(rng):
            for t in rng:
                nc.sync.dma_start(
                    out=keep_sb[:, t * 1024 : (t + 1) * 1024], in_=mkd[t]
                )

        # in loop mode all mask tiles load before the loop (one-time cost,
        # constant for delta timing); in single-shot mode they are emitted
        # inside head 0 after its first compute slices, in use order
        if loop_trips:
            load_keep(range(NCH * NPAIR))

        loop_cm = tc.For_i(0, loop_trips, 1) if loop_trips else None
        if loop_cm is not None:
            loop_cm.__enter__()

        keep_loaded = bool(loop_trips)
        # Software pipeline: the PE queue is in-order, so if the U-matmuls of
        # group i sat directly behind the S-matmuls of group i (program
        # order), PE would stall on group i's ACT->mask chain before it could
        # start group i+1's S block. Emit S(i+1) first, then U(i), then the
        # div/out of a finished chunk: PE always has independent S work.
        pend_u = None
        pend_fin = None
        for j in range(HPC * repeat):
            j = j % HPC
            # split per-head input DMAs so the first chunk's compute starts
            # as soon as its own slice lands (deps are per-tile)
            qT_sbs = [
                qkv.tile([HN, 512], qk_dt, tag=f"q{cc}", name=f"q_{cc}")
                for cc in range(NCH)
            ]
            kT_sbs = [
                qkv.tile([HN, SK // 2], qk_dt, tag=f"k{h}", name=f"k_{h}")
                for h in range(2)
            ]
            vp_sbs = [
                qkv.tile([128, (NKT // 2) * VW], bf16, tag=f"v{h}", name=f"v_{h}")
                for h in range(2)
            ]
            nc.sync.dma_start(out=qT_sbs[0], in_=qTd[j][:, 0:512])
            nc.sync.dma_start(out=kT_sbs[0], in_=kTd[j][:, 0 : SK // 2])
            nc.sync.dma_start(out=vp_sbs[0], in_=vpd[j][:, 0 : (NKT // 2) * VW])
            if not keep_loaded:
                load_keep(range(NPAIR))
            for cc in range(1, NCH):
                nc.sync.dma_start(out=qT_sbs[cc], in_=qTd[j][:, cc * 512 : (cc + 1) * 512])
            nc.sync.dma_start(out=kT_sbs[1], in_=kTd[j][:, SK // 2 : SK])
            nc.sync.dma_start(out=vp_sbs[1], in_=vpd[j][:, (NKT // 2) * VW :])
            if not keep_loaded:
                keep_loaded = True
                load_keep(range(NPAIR, NCH * NPAIR))

            for c in range(NCH):
                # matmul start=True clears the WHOLE PSUM bank's has_written
                # bits. pack_u packs two 129-col accumulation groups per
                # bank: start=True only on the bank's FIRST matmul (jj=0 /
                # jj=2 at k=0); the other group's first matmul uses
                # start=False and relies on the cleared per-element
                # has_written bits to write-instead-of-accumulate.
                if pack_u:
                    ua = upsum.tile([128, 260], f32, tag="u", name=f"ua_{c}")
                    ub = upsum.tile([128, 260], f32, tag="u", name=f"ub_{c}")
                    subs = [ua[:, 0:129], ua[:, 130:259], ub[:, 0:129], ub[:, 130:259]]
                    opens_bank = (True, False, True, False)
                else:
                    subs = [
                        upsum.tile([128, 129], f32, tag="u", name=f"u_{c}_{jj}")
                        for jj in range(4)
                    ]
                    opens_bank = (True, True, True, True)

                def emit_fin(c=c, j=j, subs=subs):
                    for jj in range(4):
                        ut = subs[jj]
                        o_sb = opool.tile([128, HN], f32)
                        if out_div == "pool":
                            nc.gpsimd.tensor_scalar(
                                o_sb,
                                in0=ut[:, 0:128],
                                scalar1=ut[:, 128:129],
                                scalar2=None,
                                op0=mybir.AluOpType.divide,
                            )
                        else:
                            rcp = rpool.tile([128, 1], f32)
                            nc.vector.reciprocal(rcp, ut[:, 128:129])
                            nc.vector.tensor_scalar_mul(
                                o_sb, in0=ut[:, 0:128], scalar1=rcp
                            )
                        r0 = c * 512 + jj * 128
                        nc.sync.dma_start(out=outd[r0 : r0 + 128, j, :], in_=o_sb)

                k0 = 0
                while k0 < NKT:
                    g = min(k_group, NKT - k0)
                    s_ps = spsum.tile([128, 512 * k_group], f32, name="s_ps")
                    for t in range(g):
                        k = k0 + t
                        kh, kr = divmod(k, NKT // 2)
                        nc.tensor.matmul(
                            s_ps[:, t * 512 : (t + 1) * 512],
                            lhsT=kT_sbs[kh][:, kr * 128 : (kr + 1) * 128],
                            rhs=qT_sbs[c],
                            start=True,
                            stop=True,
                        )
                    if pend_u is not None:
                        pend_u()
                        pend_u = None
                    if pend_fin is not None:
                        pend_fin()
                        pend_fin = None
                    pT = ppool.tile([128, 512 * k_group], bf16, name="pT")
                    n_act = int(512 * g * diag_act_frac)
                    if n_act:
                        act_src = (
                            keep_sb[:, 0:n_act] if diag_exp_const
                            else s_ps[:, :n_act]
                        )
                        nc.scalar.activation(pT[:, :n_act], act_src, Exp)
                    pmt = mpool.tile([128, 512 * k_group], bf16, name="pmt")
                    # keep_sb columns for chunk c are k-major 512-blocks
                    kc0 = c * NPAIR * 1024 + k0 * 512
                    # split the mask multiply columns between DVE and GPSIMD
                    gp = int(round(g * mask_pool_frac))
                    gd = g - gp
                    if diag_no_mask:
                        gp = gd = 0
                    if gd:
                        nc.vector.tensor_mul(
                            pmt[:, : 512 * gd],
                            pT[:, : 512 * gd],
                            keep_sb[:, kc0 : kc0 + 512 * gd],
                        )
                    if gp:
                        nc.gpsimd.tensor_mul(
                            pmt[:, 512 * gd : 512 * g],
                            pT[:, 512 * gd : 512 * g],
                            keep_sb[:, kc0 + 512 * gd : kc0 + 512 * g],
                        )

                    def emit_u(
                        k0=k0, g=g, subs=subs,
                        psrc=(pT if diag_no_mask else pmt),
                        vp_sbs=vp_sbs, opens_bank=opens_bank,
                    ):
                        for t in range(g):
                            k = k0 + t
                            kh, kr = divmod(k, NKT // 2)
                            for jj in range(4):
                                nc.tensor.matmul(
                                    subs[jj][:, 0:129],
                                    lhsT=psrc[
                                        :, t * 512 + jj * 128 : t * 512 + (jj + 1) * 128
                                    ],
                                    rhs=vp_sbs[kh][:, kr * VW : kr * VW + 129],
                                    start=(k == 0 and opens_bank[jj]),
                                    stop=(k == NKT - 1),
                                    skip_group_check=True,
                                )

                    if not diag_no_u:
                        pend_u = emit_u
                    k0 += g
                pend_fin = emit_fin
        if pend_u is not None:
            pend_u()
        if pend_fin is not None:
            pend_fin()
        if loop_cm is not None:
            loop_cm.__exit__(None, None, None)
    nc.finalize()
    return nc


def _prep_inputs(query, key, value, attention_mask):
    """Build the 8 per-core input maps (host-side layout transforms)."""
    bf16 = ml_dtypes.bfloat16
    scale = np.float32(1.0 / math.sqrt(HN))

    # [sq, b, nh, hn] -> [b, nh, hn, sq]
    qT_all = np.ascontiguousarray(query.transpose(1, 2, 3, 0)) * scale
    kT_all = np.ascontiguousarray(key.transpose(1, 2, 3, 0))
    if QK_BF16:
        qT_all = qT_all.astype(bf16)
        kT_all = kT_all.astype(bf16)

    # V' = [V | 1 | 0] per head: [b, nh, sk, 130], then DMA layout
    # [b, nh, p, k*130+c] with sk = k*128 + p
    v_all = value.transpose(1, 2, 0, 3)  # [b, nh, sk, hn]
    vp_all = np.zeros((B, NH, SK, VW), dtype=bf16)
    vp_all[:, :, :, :HN] = v_all.astype(bf16)
    vp_all[:, :, :, HN] = bf16(1.0)
    vp_dma = np.ascontiguousarray(
        vp_all.reshape(B, NH, NKT, 128, VW).transpose(0, 1, 3, 2, 4)
    ).reshape(B, NH, 128, NKT * VW)

    # keep^T in [chunk, pair, part, (jslot, u)] DMA layout, one per batch
    keeps = []
    for b in range(B):
        keepT = (~attention_mask[b, 0]).T.astype(bf16)  # [sk, sq]
        arr = keepT.reshape(NPAIR, 2, 128, NCH, 512).transpose(3, 0, 2, 1, 4)
        keeps.append(np.ascontiguousarray(arr).reshape(NCH * NPAIR, 128, 1024))

    in_maps = []
    for c in range(NCORES):
        b = c // (NCORES // B)
        h0 = (c % (NCORES // B)) * HPC
        in_maps.append(
            {
                "qT": np.ascontiguousarray(qT_all[b, h0 : h0 + HPC]),
                "kT": np.ascontiguousarray(kT_all[b, h0 : h0 + HPC]),
                "vp": np.ascontiguousarray(vp_dma[b, h0 : h0 + HPC]),
                "keep": keeps[b],
            }
        )
    return in_maps


def kernel(query, key, value, attention_mask):
    global LAST_RESULTS
    from concourse.bass_utils import run_bass_kernel_spmd

    query = np.asarray(query, dtype=np.float32)
    key = np.asarray(key, dtype=np.float32)
    value = np.asarray(value, dtype=np.float32)
    attention_mask = np.asarray(attention_mask)

    repeat = int(os.environ.get("KERNEL_REPEAT", "1"))
    if ("nc", repeat) not in _CACHE:
        _CACHE[("nc", repeat)] = _build_bass(repeat, qk_bf16=QK_BF16)
    nc = _CACHE[("nc", repeat)]

    in_maps = _prep_inputs(query, key, value, attention_mask)
    res = run_bass_kernel_spmd(nc, in_maps, core_ids=list(range(NCORES)))
    LAST_RESULTS = res

    out_full = np.empty((SQ, B, NH, HN), dtype=np.float32)
    for c in range(NCORES):
        b = c // (NCORES // B)
        h0 = (c % (NCORES // B)) * HPC
        out_full[:, b, h0 : h0 + HPC, :] = res.results[c]["out"]
    return out_full.reshape(SQ, B, NH * HN)

